# revision 29
# baseline (speedup 1.0000x reference)
"""Trainium2 Bass kernel for the NonLinearTransitionModel neural-ODE.

z_{t+1} = z_t + (dt/NSTEPS) * (tanh([z_t; u] @ W1 + b1) @ W2 + b2), 20 steps.

Sharding: data-parallel over the batch dim (8192 -> 8 x 1024), MLP weights
replicated. Per core the activations are feature-major (features on SBUF
partitions, batch on the free axis) so the mm1 -> tanh -> mm2 chain needs
no per-step transposes; batch is transposed once on entry and once on exit
via PE-transposes.

v7 design (per-step, per-core, BC=512, 2 chunks, b1 == 0 fast path):
  PE    : per chunk 8 mm1 (f32r) + 8 mm2 (bf16) N=512 matmuls. The
          constant u-contribution cu = W1u^T u is preloaded into each
          PSUM bank while it is free; mm1 accumulates onto it with
          start=False (hardware has_written bits survive).
  PSUM  : banks 0-1 of each chunk are one [128,1024] double tile (one
          fused tanh + one fused preload); banks 2-3 are singles and
          also serve as the mm2 accumulators (early-freed banks get the
          early A0/A1 deadlines, late-freed ones the late A2/A3).
  ACT   : 2 fused + 4 single tanh (bf16 out, feeds mm2) + the two
          loose-deadline preloads of chunk 1.
  DVE   : 4 hb-muls (PSUM src), the z1 adds, chunk-0 preloads.
  GpSimd: the z0 adds (GPSIMD cannot touch PSUM, so no preloads here).
All matmul operands carry their dtype tag from their producers; no
mirror copies exist anywhere.
"""

import sys

try:
    import concourse.bass as bass
except ImportError:
    sys.path.insert(0, "/opt/trn_rl_repo")
    import concourse.bass as bass

import numpy as np
import concourse.bacc as bacc
import concourse.mybir as mybir
from concourse import masks, tile
from concourse.bass_utils import run_bass_kernel_spmd

AFT = mybir.ActivationFunctionType
F32 = mybir.dt.float32
R = mybir.dt.float32r
BF16 = mybir.dt.bfloat16

N_CORES = 8
NSTEPS = 20
B, LATENT, U, HIDDEN = 8192, 256, 16, 512
BL = B // N_CORES          # batch rows per core
BC = 512                   # batch columns per chunk (free axis)
NCHUNK = BL // BC          # 2
KIN = LATENT + U           # 272
ML = LATENT // 128         # 2
MH = HIDDEN // 128         # 4

_cache = {}


def _build(add_eng="gdgd", mm2_dt="bf16", fuse_tanh=True, b2_nonzero=False,
           nwarm=20):
    opts = add_eng[4:]
    step0_gp = "0" in opts      # spread step-0's cold z-chain over GP
    act_pre3 = "+" in opts      # (c1,3) preload on ACT — measured regression
    add_base = add_eng[:4]
    nc = bacc.Bacc(None, target_bir_lowering=False, debug=False)

    zt_d = nc.dram_tensor("zt", [BL, LATENT], R, kind="ExternalInput")
    dt_d = nc.dram_tensor("dt", [BL, 1], F32, kind="ExternalInput")
    ut_d = nc.dram_tensor("ut", [BL, U], R, kind="ExternalInput")
    w1_d = nc.dram_tensor("W1", [KIN, HIDDEN], R, kind="ExternalInput")
    b1_d = nc.dram_tensor("b1", [HIDDEN], F32, kind="ExternalInput")
    w2_d = nc.dram_tensor("W2", [HIDDEN, LATENT], R, kind="ExternalInput")
    b2_d = nc.dram_tensor("b2", [LATENT], F32, kind="ExternalInput")
    out_d = nc.dram_tensor("out", [BL, LATENT], F32, kind="ExternalOutput")

    M2 = BF16 if mm2_dt == "bf16" else R

    with tile.TileContext(nc) as tc:
        with (
            tc.tile_pool(name="const", bufs=1) as cpool,
            tc.tile_pool(name="state", bufs=1) as spool,
            tc.tile_pool(name="stage", bufs=1) as gpool,
            tc.tile_pool(name="tbuf", bufs=4) as tpool,
            tc.tile_pool(name="obuf", bufs=4) as opool,
            tc.tile_pool(name="psum", bufs=1, space="PSUM") as ppool,
        ):
            # -------- PSUM: per chunk one double tile (banks 0,1) + two
            # single-bank tiles (banks 2,3; also the mm2 accumulators) ----
            pbD = []
            pb23 = []
            for c in range(NCHUNK):
                d = ppool.tile([128, 2 * BC], F32, tag=f"pbD_{c}", name=f"pbD_{c}")
                s2 = ppool.tile([128, BC], F32, tag=f"pb2_{c}", name=f"pb2_{c}")
                s3 = ppool.tile([128, BC], F32, tag=f"pb3_{c}", name=f"pb3_{c}")
                pbD.append(d)
                pb23.append([s2, s3])

            def bankv(c, m):
                """AP view of mm1 bank m of chunk c."""
                if m < 2:
                    return pbD[c][:, m * BC : (m + 1) * BC]
                return pb23[c][m - 2][:]

            # -------- constants / table warm-up --------
            ident_f = cpool.tile([128, 128], F32, tag="ident_f")
            masks.make_identity(nc, ident_f[:])
            identR = cpool.tile([128, 128], R, tag="identR")
            nc.sync.dma_start(identR[:], ident_f[:].bitcast(R))
            # load the ACT tanh table while DMAs are in flight
            wup = cpool.tile([128, 128], F32, tag="wup")
            nc.scalar.activation(wup[:], ident_f[:], AFT.Tanh)

            # -------- input DMAs, few and large --------
            # zt halves split across the two HWDGE rings so both land early;
            # the weights follow behind them (w2 is not needed until mm2).
            zbs = []
            for c in range(NCHUNK):
                zbc = gpool.tile([128, 4 * LATENT], R, tag=f"zb{c}", name=f"zb{c}")
                eng = nc.sync if c == 0 else nc.scalar
                eng.dma_start(
                    zbc[:].rearrange("p (c l) -> p c l", l=LATENT),
                    zt_d.ap()[c * BC : (c + 1) * BC, :].rearrange(
                        "(c p) l -> p c l", p=128
                    ),
                )
                zbs.append(zbc)
            w1ab = cpool.tile([128, 2 * HIDDEN], R, tag="w1ab")
            nc.scalar.dma_start(
                w1ab[:].rearrange("p (k h) -> p k h", h=HIDDEN),
                w1_d.ap()[0 : 2 * 128, :].rearrange("(k p) h -> p k h", p=128),
            )
            w2all = cpool.tile([128, MH * LATENT], F32, tag="w2all")
            nc.sync.dma_start(
                w2all[:].bitcast(R).rearrange("p (k l) -> p k l", l=LATENT),
                w2_d.ap().rearrange("(k p) l -> p k l", p=128),
            )
            w1u = cpool.tile([U, HIDDEN], R, tag="w1u")
            nc.gpsimd.dma_start(w1u[:], w1_d.ap()[2 * 128 : KIN, :])
            ub = gpool.tile([128, NCHUNK * 4 * U], R, tag="ub", name="ub")
            nc.gpsimd.dma_start(
                ub[:].rearrange("p (c u) -> p c u", u=U),
                ut_d.ap().rearrange("(c p) u -> p c u", p=128),
            )
            b1t = cpool.tile([128, MH], F32, tag="b1t")
            nc.gpsimd.dma_start(b1t[:], b1_d.ap().rearrange("(f p) -> p f", p=128))
            h_row = cpool.tile([1, BL], F32, tag="hrow")
            nc.gpsimd.dma_start(h_row[:], dt_d.ap().rearrange("b o -> o b"))
            if b2_nonzero:
                b2f = cpool.tile([1, LATENT], F32, tag="b2f")
                nc.gpsimd.dma_start(b2f[:], b2_d.ap().unsqueeze(0))
                b2r = cpool.tile([1, LATENT], M2, tag="b2r")
                nc.vector.tensor_copy(b2r[:], b2f[:])
                onesb_f = cpool.tile([1, BC], F32, tag="onesb_f")
                nc.vector.memset(onesb_f[:], 1.0)
                onesb = cpool.tile([1, BC], M2, tag="onesb")
                nc.vector.tensor_copy(onesb[:], onesb_f[:])

            w1a = w1ab[:, 0:HIDDEN]
            w1b = w1ab[:, HIDDEN : 2 * HIDDEN]
            if mm2_dt == "bf16":
                w2m = cpool.tile([128, MH * LATENT], BF16, tag="w2m")
                nc.vector.tensor_copy(w2m[:], w2all[:])
                w2t = [w2m[:, k * LATENT : (k + 1) * LATENT] for k in range(MH)]
            else:
                w2t = [
                    w2all[:, k * LATENT : (k + 1) * LATENT].bitcast(R)
                    for k in range(MH)
                ]

            # -------- PE warm-up stream (HAM) while DMAs land --------
            for i in range(nwarm):
                nc.tensor.transpose(
                    bankv(0, 0)[:, 0:128].bitcast(R), identR[:], identR[:]
                )

            # -------- z transposes: zb -> banks 0,1 -> zts --------
            zts = [
                [
                    spool.tile([128, BC], R, tag=f"z_{c}_{l}", name=f"z_{c}_{l}")
                    for l in range(ML)
                ]
                for c in range(NCHUNK)
            ]
            for c in range(NCHUNK):
                for l in range(ML):
                    bank = bankv(c, l)
                    for j in range(BC // 128):
                        nc.tensor.transpose(
                            bank[:, j * 128 : (j + 1) * 128].bitcast(R),
                            zbs[c][:, j * LATENT + l * 128 : j * LATENT + (l + 1) * 128],
                            identR[:],
                        )
                    if (c + l) % 2 == 0:
                        nc.scalar.activation(zts[c][l][:], bank, AFT.Copy)
                    else:
                        nc.vector.tensor_copy(zts[c][l][:], bank)

            # -------- u transposes: ub -> bank (c,2) low partitions -> uts ----
            uts = []
            for c in range(NCHUNK):
                bank = bankv(c, 2)
                for j in range(BC // 128):
                    ci = c * (BC // 128) + j
                    nc.tensor.transpose(
                        bank[0:U, j * 128 : (j + 1) * 128].bitcast(R),
                        ub[:, ci * U : (ci + 1) * U],
                        identR[:],
                    )
                ut_c = spool.tile([U, BC], R, tag=f"ut_{c}", name=f"ut_{c}")
                nc.vector.tensor_copy(ut_c[:], bank[0:U, :])
                uts.append(ut_c)

            # -------- hb[p, b] = dt[b] / NSTEPS, replicated over partitions ----
            h_sc = cpool.tile([1, BL], F32, tag="h_sc")
            nc.scalar.activation(h_sc[:], h_row[:], AFT.Copy, scale=1.0 / NSTEPS)
            hb = cpool.tile([128, BL], F32, tag="hb")
            nc.gpsimd.partition_broadcast(hb[:], h_sc[:])

            # -------- cu precompute (primes every bank for t=0) --------
            cuD = []
            cu23 = []
            for c in range(NCHUNK):
                for m in range(MH):
                    nc.tensor.matmul(
                        bankv(c, m),
                        w1u[:, bass.ts(m, 128)],
                        uts[c][:],
                        start=True,
                        stop=True,
                    )
                cD = spool.tile([128, 2 * BC], F32, tag=f"cuD_{c}", name=f"cuD_{c}")
                nc.scalar.activation(cD[:], pbD[c][:], AFT.Copy)
                cuD.append(cD)
                row = []
                for m in (2, 3):
                    cu = spool.tile(
                        [128, BC], F32, tag=f"cu_{c}_{m}", name=f"cu_{c}_{m}"
                    )
                    nc.vector.tensor_copy(cu[:], bankv(c, m))
                    row.append(cu)
                cu23.append(row)

            # ht: double tile for m0/m1 (fused tanh target) + singles
            htD = [
                spool.tile([128, 2 * BC], M2, tag=f"hD_{c}", name=f"hD_{c}")
                for c in range(NCHUNK)
            ]
            ht23 = [
                [
                    spool.tile([128, BC], M2, tag=f"h_{c}_{m}", name=f"h_{c}_{m}")
                    for m in (2, 3)
                ]
                for c in range(NCHUNK)
            ]

            def htv(c, k):
                if k < 2:
                    return htD[c][:, k * BC : (k + 1) * BC]
                return ht23[c][k - 2][:]

            # ---------------- main loop ----------------
            for t in range(NSTEPS):
                last = t == NSTEPS - 1
                for c in range(NCHUNK):
                    for m in range(MH):
                        nc.tensor.matmul(
                            bankv(c, m),
                            w1a[:, bass.ts(m, 128)],
                            zts[c][0][:],
                            start=False,
                            stop=False,
                            skip_group_check=True,
                        )
                    for m in range(MH):
                        nc.tensor.matmul(
                            bankv(c, m),
                            w1b[:, bass.ts(m, 128)],
                            zts[c][1][:],
                            start=False,
                            stop=True,
                            skip_group_check=True,
                        )
                    if fuse_tanh:
                        nc.scalar.activation(htD[c][:], pbD[c][:], AFT.Tanh)
                        for m in (2, 3):
                            nc.scalar.activation(
                                ht23[c][m - 2][:], bankv(c, m), AFT.Tanh
                            )
                    else:
                        for m in range(MH):
                            nc.scalar.activation(
                                htv(c, m), bankv(c, m), AFT.Tanh,
                                bias=b1t[:, m : m + 1],
                            )
                    if not last and c == 0:
                        # chunk-0 double preload on DVE (early, no conflicts)
                        nc.vector.tensor_copy(pbD[0][:], cuD[0][:])
                if not last:
                    # chunk-1 double preload on ACT right after its tanhs;
                    # loose deadline (a whole mm2 phase + mm1(c0) away).
                    nc.scalar.activation(pbD[1][:], cuD[1][:], AFT.Copy)
                for c in range(NCHUNK):
                    cs = bass.ts(c, BC)
                    for l in range(ML):
                        p2 = pb23[c][l]
                        for k in range(MH):
                            nc.tensor.matmul(
                                p2[:],
                                w2t[k][:, bass.ts(l, 128)],
                                htv(c, k),
                                start=(k == 0),
                                stop=(k == MH - 1) and not b2_nonzero,
                            )
                        if b2_nonzero:
                            nc.tensor.matmul(
                                p2[:], b2r[:, bass.ts(l, 128)],
                                onesb[:], start=False, stop=True,
                            )
                        tmp = tpool.tile([128, BC], F32, tag="tmp", bufs=4)
                        nc.vector.tensor_mul(tmp[:], p2[:], hb[:, cs])
                        # step 0's chain is cold (no pipeline overlap yet):
                        # spread its adds over GP to unclog DVE.
                        amap = "gdgd" if (step0_gp and t == 0) else add_base
                        on_gp = (not last) and amap[c * ML + l] == "g"
                        if on_gp:
                            nc.gpsimd.tensor_add(
                                zts[c][l][:], zts[c][l][:].bitcast(F32), tmp[:]
                            )
                        else:
                            nc.vector.tensor_add(
                                zts[c][l][:], zts[c][l][:].bitcast(F32), tmp[:]
                            )
                        if not last:
                            # late-freed mm2 bank: (c1,2) — and with tweaks
                            # (c1,3) — on ACT (loose deadlines, keeps the
                            # DVE tail short for the next step's hoisted
                            # matmul wait); the rest on DVE.
                            on_act = c == 1 and (l == 0 or act_pre3)
                            if on_act:
                                nc.scalar.activation(
                                    p2[:], cu23[c][l][:], AFT.Copy
                                )
                            else:
                                nc.vector.tensor_copy(p2[:], cu23[c][l][:])

            # ---------------- epilogue: transpose back, store ----------------
            for c in range(NCHUNK):
                for half in range(2):
                    bank = bankv(c, half)
                    for jj in range(2):
                        j = half * 2 + jj
                        for l in range(ML):
                            nc.tensor.transpose(
                                bank[:, (jj * ML + l) * 128 : (jj * ML + l + 1) * 128].bitcast(R),
                                zts[c][l][:, j * 128 : (j + 1) * 128],
                                identR[:],
                            )
                    zo = opool.tile([128, 2 * LATENT], F32, tag="zo", bufs=4)
                    if half == 0:
                        nc.scalar.activation(zo[:], bank, AFT.Copy)
                    else:
                        nc.vector.tensor_copy(zo[:], bank)
                    r0 = (c * 4 + half * 2) * 128
                    nc.sync.dma_start(
                        out_d.ap()[r0 : r0 + 256, :].rearrange(
                            "(two p) l -> p two l", p=128
                        ),
                        zo[:].rearrange("p (two l) -> p two l", l=LATENT),
                    )

    nc.compile()
    return nc


def _get_nc(add_eng, mm2_dt, fuse_tanh, b2_nonzero):
    key = (add_eng, mm2_dt, fuse_tanh, b2_nonzero)
    if key not in _cache:
        _cache[key] = _build(add_eng, mm2_dt, fuse_tanh, b2_nonzero)
    return _cache[key]


def _run(inputs, add_eng="gdgd", mm2_dt="bf16", trace=False):
    zt = np.ascontiguousarray(inputs["zt"], dtype=np.float32)
    dt = np.ascontiguousarray(inputs["dt"], dtype=np.float32)
    ut = np.ascontiguousarray(inputs["ut"], dtype=np.float32)
    W1 = np.ascontiguousarray(inputs["W1"], dtype=np.float32)
    b1 = np.ascontiguousarray(inputs["b1"], dtype=np.float32)
    W2 = np.ascontiguousarray(inputs["W2"], dtype=np.float32)
    b2 = np.ascontiguousarray(inputs["b2"], dtype=np.float32)

    b2_nonzero = bool(np.any(b2))
    fuse_tanh = not bool(np.any(b1))
    nc = _get_nc(add_eng, mm2_dt, fuse_tanh, b2_nonzero)

    in_maps = []
    for i in range(N_CORES):
        sl = slice(i * BL, (i + 1) * BL)
        in_maps.append(
            {
                "zt": zt[sl],
                "dt": dt[sl],
                "ut": ut[sl],
                "W1": W1,
                "b1": b1,
                "W2": W2,
                "b2": b2,
            }
        )
    res = run_bass_kernel_spmd(nc, in_maps, list(range(N_CORES)), trace=trace)
    out = np.concatenate([res.results[i]["out"] for i in range(N_CORES)], axis=0)
    return out, res


def kernel(**inputs):
    out, _ = _run(inputs, add_eng="gddd", mm2_dt="bf16")
    return out
